# revision 1
# baseline (speedup 1.0000x reference)
import numpy as np
import jax
import jax.numpy as jnp

# nn_GATop: GAT-style message passing. Hardcoded problem shapes.
B, NOP, NMA = 16, 512, 64
D_OP, D_MA, D_EG = 16, 8, 8
HID, OUT = 256, 128
NEG_SLOPE = 0.2
NDEV = 8  # data-parallel over batch: 16 / 8 = 2 instances per core


def _mlp(x, w1, b1, w2, b2, w3, b3):
    h = jax.nn.elu(x @ w1 + b1)
    h = jax.nn.elu(h @ w2 + b2)
    return h @ w3 + b3


def _gatop_shard(op, ma, eg, ma_adj, pre_adj, sub_adj, attn_pre, *ws):
    params = [tuple(ws[i * 6:(i + 1) * 6]) for i in range(5)]
    num = op.shape[1]
    pre_f1 = _mlp(jnp.einsum('bij,bjd->bid', pre_adj, op), *params[1])
    sub_f1 = _mlp(jnp.einsum('bij,bjd->bid', sub_adj, op), *params[2])
    self_f1 = _mlp(op, *params[3])
    ma_eg = jnp.concatenate([ma_adj[..., None] * ma[:, None, :, :], eg], axis=-1)
    ma_f1 = _mlp(ma_adj[..., None] * ma_eg, *params[0])
    cnt = jnp.sum(ma_f1 != 0.0, axis=-2).astype(ma_f1.dtype)
    ma_f2 = _mlp(jnp.sum(ma_f1, axis=-2) / cnt, *params[4])
    att = lambda f: jnp.sum(f * attn_pre, axis=-1, keepdims=True)
    pre_ut, sub_ut, self_ut, ma_ut = att(pre_f1), att(sub_f1), att(self_f1), att(ma_f2)
    lrelu = lambda x: jax.nn.leaky_relu(x, NEG_SLOPE)
    scores = jnp.concatenate([
        lrelu(pre_ut + self_ut), lrelu(sub_ut + self_ut),
        lrelu(self_ut + self_ut), lrelu(ma_ut + self_ut)], axis=1)
    alpha = jax.nn.softmax(scores, axis=1)
    a_pre = alpha[:, :num]
    a_sub = alpha[:, num:2 * num]
    a_self = alpha[:, 2 * num:3 * num]
    a_ma = alpha[:, 3 * num:]
    return jax.nn.sigmoid(a_pre * pre_f1 + a_sub * sub_f1 + a_self * self_f1 + a_ma * ma_f2)


# first 6 args sharded on batch, attn_pre + 30 weight tensors replicated
_PMAPPED = jax.pmap(_gatop_shard, in_axes=(0, 0, 0, 0, 0, 0) + (None,) * 31)


def _shard(x):
    return x.reshape((NDEV, B // NDEV) + x.shape[1:])


def kernel(**inputs) -> np.ndarray:
    bidx = np.asarray(inputs['batch_idxes'])
    op = np.asarray(inputs['op'], np.float32)
    ma = np.asarray(inputs['ma'], np.float32)
    eg = np.asarray(inputs['eg'], np.float32)
    ma_adj = np.asarray(inputs['op_ma_adj'])[bidx].astype(np.float32)
    pre_adj = np.asarray(inputs['op_pre_adj'])[bidx].astype(np.float32)
    sub_adj = np.asarray(inputs['op_sub_adj'])[bidx].astype(np.float32)
    attn_pre = np.asarray(inputs['attn_pre'], np.float32)
    ws = []
    for i in range(5):
        for p in ('w1', 'b1', 'w2', 'b2', 'w3', 'b3'):
            ws.append(np.asarray(inputs[f'm{i}_{p}'], np.float32))
    out = _PMAPPED(_shard(op), _shard(ma), _shard(eg), _shard(ma_adj),
                   _shard(pre_adj), _shard(sub_adj), attn_pre, *ws)
    return np.asarray(out).reshape(B, NOP, OUT)


# revision 3
# speedup vs baseline: 1.1671x; 1.1671x over previous
import numpy as np
import jax
import jax.numpy as jnp

# nn_GATop: GAT-style message passing. Hardcoded problem shapes.
B, NOP, NMA = 16, 512, 64
D_OP, D_MA, D_EG = 16, 8, 8
HID, OUT = 256, 128
NEG_SLOPE = 0.2
NDEV = 8  # data-parallel over batch: 16 / 8 = 2 instances per core


def _mlp(x, w1, b1, w2, b2, w3, b3):
    h = jax.nn.elu(x @ w1 + b1)
    h = jax.nn.elu(h @ w2 + b2)
    return h @ w3 + b3


def _gatop_shard(op, ma, eg, ma_adj, pre_adj, sub_adj, attn_pre, *ws):
    params = [tuple(ws[i * 6:(i + 1) * 6]) for i in range(5)]
    num = op.shape[1]
    # adjacencies arrive as int8 (4x fewer bytes over the tunnel); exact 0/1
    ma_adj = ma_adj.astype(jnp.float32)
    pre_adj = pre_adj.astype(jnp.float32)
    sub_adj = sub_adj.astype(jnp.float32)
    pre_f1 = _mlp(jnp.einsum('bij,bjd->bid', pre_adj, op), *params[1])
    sub_f1 = _mlp(jnp.einsum('bij,bjd->bid', sub_adj, op), *params[2])
    self_f1 = _mlp(op, *params[3])
    ma_eg = jnp.concatenate([ma_adj[..., None] * ma[:, None, :, :], eg], axis=-1)
    ma_f1 = _mlp(ma_adj[..., None] * ma_eg, *params[0])
    cnt = jnp.sum(ma_f1 != 0.0, axis=-2).astype(ma_f1.dtype)
    ma_f2 = _mlp(jnp.sum(ma_f1, axis=-2) / cnt, *params[4])
    att = lambda f: jnp.sum(f * attn_pre, axis=-1, keepdims=True)
    pre_ut, sub_ut, self_ut, ma_ut = att(pre_f1), att(sub_f1), att(self_f1), att(ma_f2)
    lrelu = lambda x: jax.nn.leaky_relu(x, NEG_SLOPE)
    scores = jnp.concatenate([
        lrelu(pre_ut + self_ut), lrelu(sub_ut + self_ut),
        lrelu(self_ut + self_ut), lrelu(ma_ut + self_ut)], axis=1)
    alpha = jax.nn.softmax(scores, axis=1)
    a_pre = alpha[:, :num]
    a_sub = alpha[:, num:2 * num]
    a_self = alpha[:, 2 * num:3 * num]
    a_ma = alpha[:, 3 * num:]
    return jax.nn.sigmoid(a_pre * pre_f1 + a_sub * sub_f1 + a_self * self_f1 + a_ma * ma_f2)


# first 6 args sharded on batch, attn_pre + 30 weight tensors replicated
_PMAPPED = jax.pmap(_gatop_shard, in_axes=(0, 0, 0, 0, 0, 0) + (None,) * 31)


def _shard(x):
    return x.reshape((NDEV, B // NDEV) + x.shape[1:])


def kernel(**inputs) -> np.ndarray:
    bidx = np.asarray(inputs['batch_idxes'])
    op = np.asarray(inputs['op'], np.float32)
    ma = np.asarray(inputs['ma'], np.float32)
    eg = np.asarray(inputs['eg'], np.float32)
    ma_adj = np.asarray(inputs['op_ma_adj'])[bidx].astype(np.int8)
    pre_adj = np.asarray(inputs['op_pre_adj'])[bidx].astype(np.int8)
    sub_adj = np.asarray(inputs['op_sub_adj'])[bidx].astype(np.int8)
    attn_pre = np.asarray(inputs['attn_pre'], np.float32)
    ws = []
    for i in range(5):
        for p in ('w1', 'b1', 'w2', 'b2', 'w3', 'b3'):
            ws.append(np.asarray(inputs[f'm{i}_{p}'], np.float32))
    out = _PMAPPED(_shard(op), _shard(ma), _shard(eg), _shard(ma_adj),
                   _shard(pre_adj), _shard(sub_adj), attn_pre, *ws)
    return np.asarray(out).reshape(B, NOP, OUT)
